# revision 8
# baseline (speedup 1.0000x reference)
"""DoubleAttention forward on 8 Trainium2 NeuronCores.

Reference (per sample, x: [512, 4096] after flattening h*w):
    A = wA @ x + bA            [128, n]
    B = wB @ x + bB            [128, n]
    V = wV @ x + bV            [128, n]
    M = softmax(B, axis=ch)    [128, n]
    W = softmax(V, axis=ch)    [128, n]
    gd = A @ M.T               [128, 128]
    Z = gd @ W                 [128, n]
    out = wR @ Z + bR          [512, n]

Sharding: data-parallel over batch, 16 samples -> 8 cores x 2 each.

Schedule (all matmuls fp16; PE stream floor ~62us at 2.4GHz):
  - A|B projections per 128-pos tile in [pos, ch] layout (needed by the
    gd contraction); x chunks stationary, 256-col streams.
  - V^T computed directly in [ch, pos] layout (weight chunks stationary,
    512-col streams) so phase 3 needs NO PE transposes: out = G @ W^T.
    Softmax-V partition sums come from a ones128 matmul (sums replicated
    across partitions), reciprocal+normalize on DVE at fp16.
  - softmax-B folded into the A-tile evacuation (per-partition scale).
  - gd accumulates on PE across pairs with a 2-chunk emission lag.
  - out = (wR gd) @ Wn: G^T = gd^T @ wR^T (one 512-col matmul), then 32
    direct 512-col output matmuls per sample.
  - DMA: host pre-packs x/out so every transfer is [128 part x 4KB
    contiguous]; 512KB chunks. Loads issued up-front on the sync HWDGE
    queue; stores issued per-chunk on the scalar HWDGE queue.
  - PSUM: pairs 1 bank x3, Vt 1 bank x2, gd 1, out 2 -- out matmuls
    rotate over 5 psum slots (pO+pV+pG) so evacuation never stalls PE.
  - Samples interleaved at phase boundaries to keep PE dense (p-state).
  - Elementwise split: scalar=exps+gts+1 evac lane, DVE=reduce/recip/
    Wn+1 evac lane, gpsimd=A-scale+1-2 evac lanes.
"""

import sys

if "/opt/trn_rl_repo" not in sys.path:
    sys.path.insert(0, "/opt/trn_rl_repo")

import numpy as np

import concourse.bacc as bacc
import concourse.tile as tile
from concourse import mybir
from concourse.bass_utils import run_bass_kernel_spmd

N_CORES = 8
B_GLOBAL = 16
B_LOC = B_GLOBAL // N_CORES
C_IN, C_M, C_N = 512, 128, 128
H = W = 64
N = H * W                      # 4096 spatial positions
KC = C_IN // 128               # 4 contraction chunks
NG = 8                         # 8 n-chunks of 512 positions
NP = 16                        # 16 tile-pairs (2 x 128 pos) per sample
SHIFT = float(-12.0 * np.log(2.0))   # exp downshift so fp16 never overflows
F32 = mybir.dt.float32
F16 = mybir.dt.float16
EXP = mybir.ActivationFunctionType.Exp
IDENT = mybir.ActivationFunctionType.Identity
AXX = mybir.AxisListType.X


def _build(has_bias_abv: bool, has_bias_r: bool):
    nc = bacc.Bacc("TRN2", target_bir_lowering=False, debug=False)

    # x / out packed host-side as [s, g, p, k, n']:  [p][k*512+n'] rows are
    # 4KB-contiguous per partition -> 128-descriptor DMAs.
    x_d = nc.dram_tensor("x", (B_LOC, NG, 128, KC, 512), F16, kind="ExternalInput")
    wab_d = nc.dram_tensor("wab", (KC, 128, 256), F16, kind="ExternalInput")
    wvt_d = nc.dram_tensor("wvt", (KC, 128, 128), F16, kind="ExternalInput")
    wrt_d = nc.dram_tensor("wrt", (128, C_IN), F16, kind="ExternalInput")
    if has_bias_abv:
        bab_d = nc.dram_tensor("bab", (1, 256), F16, kind="ExternalInput")
        bvt_d = nc.dram_tensor("bvt", (128, 1), F32, kind="ExternalInput")
    if has_bias_r:
        brt_d = nc.dram_tensor("brt", (128, KC), F32, kind="ExternalInput")
    out_d = nc.dram_tensor("out", (B_LOC, NG, 128, KC, 512), F16, kind="ExternalOutput")

    with tile.TileContext(nc) as tc:
        with (
            tc.tile_pool(name="const", bufs=1) as constp,
            tc.tile_pool(name="xq", bufs=B_LOC * NG) as xqp,
            tc.tile_pool(name="mwb", bufs=10) as mwp,
            tc.tile_pool(name="at", bufs=10) as atp,
            tc.tile_pool(name="st", bufs=8) as stp,
            tc.tile_pool(name="ev", bufs=2) as evp,
            tc.tile_pool(name="wn", bufs=2) as wnp,
            tc.tile_pool(name="rw", bufs=4) as rwp,
            tc.tile_pool(name="gds", bufs=2) as gdsp,
            tc.tile_pool(name="gts", bufs=2) as gtsp,
            tc.tile_pool(name="osb", bufs=4) as osbp,
            tc.tile_pool(name="pP", bufs=3, space="PSUM") as pP,
            tc.tile_pool(name="pV", bufs=2, space="PSUM") as pV,
            tc.tile_pool(name="pG", bufs=1, space="PSUM") as pG,
            tc.tile_pool(name="pO", bufs=2, space="PSUM") as pO,
        ):
            # ---- constants + full x prefetch (sync HWDGE queue) ----
            wab = constp.tile([128, KC, 256], F16)
            nc.sync.dma_start(wab[:], wab_d.ap().rearrange("k p j -> p k j"))
            xqs = [[None] * NG for _ in range(B_LOC)]

            def load_chunk(s, g):
                t = xqp.tile([128, KC, 512], F16, tag="xq", name=f"xq{s}_{g}")
                nc.sync.dma_start(t[:], x_d[s, g])
                xqs[s][g] = t

            load_chunk(0, 0)
            wvt = constp.tile([128, KC, 128], F16)
            nc.sync.dma_start(wvt[:], wvt_d.ap().rearrange("k p j -> p k j"))
            load_chunk(0, 1)
            wrt = constp.tile([128, C_IN], F16)
            nc.sync.dma_start(wrt[:], wrt_d[:])
            for g in range(2, NG):
                load_chunk(0, g)
            for g in range(NG):
                load_chunk(1, g)

            ones128 = constp.tile([128, 128], F16)
            nc.gpsimd.memset(ones128[:], 1.0)
            shiftv = constp.tile([128, 1], F32)
            nc.gpsimd.memset(shiftv[:], SHIFT)
            if has_bias_abv:
                bab = constp.tile([1, 256], F16)
                nc.sync.dma_start(bab[:], bab_d[:])
                ones1 = constp.tile([1, 128], F16)
                nc.gpsimd.memset(ones1[:], 1.0)
                bvt = constp.tile([128, 1], F32)
                nc.sync.dma_start(bvt[:], bvt_d[:])
                shiftbv = constp.tile([128, 1], F32)
                nc.vector.tensor_scalar_add(shiftbv[:], bvt[:], SHIFT)
                vbias = shiftbv
            else:
                vbias = shiftv
            if has_bias_r:
                brt = constp.tile([128, KC], F32)
                nc.sync.dma_start(brt[:], brt_d[:])

            # per-sample state
            mwB = [[None] * NP, [None] * NP]   # exp(B) fp16 [128, 2, 128]
            ats = [[None] * NP, [None] * NP]   # A/sumB fp16 [128, 2, 128]
            evt = [None, None]                 # exp(V^T) fp16 [128, NG, 512]
            wnt = [None, None]                 # normalized W^T fp16
            gdt = [None, None]                 # gd psum
            gts = [None, None]                 # G^T fp16 [128, 512]

            def emit_pair(s, i):
                """A|B projections for tiles 2i, 2i+1 + softmax-B chain."""
                xq = xqs[s]
                p1 = pP.tile([128, 512], F32, tag="pp", name=f"pp{s}_{i}")
                p1v = p1.rearrange("p (j a c) -> p j a c", j=2, c=128)
                for j in (0, 1):
                    nt = 2 * i + j
                    g, col = nt // 4, (nt % 4) * 128
                    dst = p1[:, j * 256:j * 256 + 256]
                    for k in range(KC):
                        nc.tensor.matmul(
                            dst, xq[g][:, k, col:col + 128], wab[:, k, :],
                            start=(k == 0),
                            stop=(k == KC - 1 and not has_bias_abv),
                        )
                    if has_bias_abv:
                        nc.tensor.matmul(dst, ones1[:], bab[:],
                                         start=False, stop=True)
                # exp(B + SHIFT) -> fp16 straight into the gd rhs store;
                # accum_out gives the channel sums for free (no DVE reduce)
                mw = mwp.tile([128, 2, 128], F16, tag="mwb", name=f"mw{s}_{i}")
                sums = stp.tile([128, 2, 1], F32, tag="sums")
                for j in (0, 1):
                    nc.scalar.activation(mw[:, j, :], p1v[:, j, 1, :], EXP,
                                         bias=shiftv[:],
                                         accum_out=sums[:, j, :])
                rec = stp.tile([128, 2, 1], F32, tag="rec")
                nc.vector.reciprocal(rec[:], sums[:])
                # A scaled by 1/sum(expB): the entire softmax-B normalization
                at = atp.tile([128, 2, 128], F16, tag="at", name=f"at{s}_{i}")
                nc.vector.tensor_mul(at[:], p1v[:, :, 0, :],
                                     rec[:].broadcast_to([128, 2, 128]))
                mwB[s][i] = mw
                ats[s][i] = at

            def emit_vt(s, g):
                """V^T [ch, pos] projection for n-chunk g + exp."""
                pv = pV.tile([128, 512], F32, tag="pv", name=f"pv{s}_{g}")
                for k in range(KC):
                    nc.tensor.matmul(pv[:], wvt[:, k, :], xqs[s][g][:, k, :],
                                     start=(k == 0), stop=(k == KC - 1))
                nc.scalar.activation(evt[s][:, g, :], pv[:], EXP, bias=vbias[:])

            def emit_vsum(s, g):
                """Partition sums of exp(V^T) via ones-matmul; recip; Wn."""
                ps = pO.tile([128, 512], F32, tag="po", name=f"ps{s}_{g}")
                nc.tensor.matmul(ps[:], ones128[:], evt[s][:, g, :],
                                 start=True, stop=True)
                rw = rwp.tile([128, 512], F16, tag="rw", name=f"rw{s}_{g}")
                with nc.allow_low_precision("softmax recip fp16 is plenty"):
                    nc.vector.reciprocal(rw[:], ps[:])
                # fp16 SBUF-only multiply: the one op gpsimd CAN run
                nc.gpsimd.tensor_mul(wnt[s][:, g, :], evt[s][:, g, :], rw[:])

            def emit_gd(s, i, first, last):
                for j in (0, 1):
                    nc.tensor.matmul(
                        gdt[s][:], ats[s][i][:, j, :], mwB[s][i][:, j, :],
                        start=(first and j == 0), stop=(last and j == 1),
                        skip_group_check=True,
                    )

            def emit_chunk(s, g):
                """All phase-1 work tied to x chunk g of sample s."""
                if g == 0:
                    evt[s] = evp.tile([128, NG, 512], F16, tag="ev",
                                      name=f"ev{s}")
                    wnt[s] = wnp.tile([128, NG, 512], F16, tag="wn",
                                      name=f"wn{s}")
                emit_pair(s, 2 * g)
                emit_pair(s, 2 * g + 1)
                emit_vt(s, g)
                if g >= 1:
                    emit_vsum(s, g - 1)
                if g >= 2:
                    if g == 2:
                        # allocated here (not at g==0) so pG's ring order
                        # matches use order across the sample interleave
                        gdt[s] = pG.tile([128, 128], F32, tag="pg",
                                         name=f"gd{s}")
                    emit_gd(s, 2 * g - 4, first=(g == 2), last=False)
                    emit_gd(s, 2 * g - 3, first=False, last=False)

            def emit_tail(s):
                """Finish gd, compute G^T = gd^T @ wR^T."""
                emit_vsum(s, NG - 1)
                for i in (NP - 4, NP - 3, NP - 2, NP - 1):
                    emit_gd(s, i, first=False, last=(i == NP - 1))
                gdts = gdsp.tile([128, 128], F16, tag="gds", name=f"gds{s}")
                nc.vector.tensor_copy(gdts[:], gdt[s][:])
                gtp = pO.tile([128, 512], F32, tag="po", name=f"gtp{s}")
                nc.tensor.matmul(gtp[:], gdts[:], wrt[:], start=True, stop=True)
                gt = gtsp.tile([128, 512], F16, tag="gts", name=f"gt{s}")
                nc.scalar.copy(gt[:], gtp[:])
                gts[s] = gt

            def emit_out(s):
                """out chunks: 4 c-block matmuls + engine-rotated evacuation
                + store per n-chunk.  5 rotating psum slots (pO/pV/pG)."""
                slot = 0
                for g in range(NG):
                    osb = osbp.tile([128, KC, 512], F16, tag="osb",
                                    name=f"osb{s}_{g}")
                    for b in range(KC):
                        pool, ptag = ((pO, "po"), (pO, "po"), (pV, "pv"),
                                      (pV, "pv"), (pG, "pg"))[slot % 5]
                        slot += 1
                        ock = pool.tile([128, 512], F32, tag=ptag,
                                        name=f"ock{s}_{g}_{b}")
                        nc.tensor.matmul(ock[:], gts[s][:, b * 128:b * 128 + 128],
                                         wnt[s][:, g, :], start=True, stop=True)
                        dst = osb[:, b, :]
                        if has_bias_r:
                            nc.scalar.activation(dst, ock[:], IDENT,
                                                 bias=brt[:, b:b + 1])
                        elif b < 2:
                            nc.scalar.copy(dst, ock[:])
                        else:
                            nc.vector.tensor_copy(dst, ock[:])
                    nc.scalar.dma_start(out_d[s, g], osb[:])

            # ---- schedule: s0 phase-1, s1 head pads s0 tail, etc. ----
            for g in range(NG):
                emit_chunk(0, g)
            emit_chunk(1, 0)          # pads s0's gd-tail wait
            emit_tail(0)
            emit_chunk(1, 1)          # pads s0's G -> gts chain
            emit_out(0)
            for g in range(2, NG):
                emit_chunk(1, g)
            emit_tail(1)
            emit_out(1)

    nc.compile()
    return nc


_CACHE = {}


def _get_nc(has_bias_abv: bool, has_bias_r: bool):
    key = (has_bias_abv, has_bias_r)
    if key not in _CACHE:
        _CACHE[key] = _build(*key)
    return _CACHE[key]


def _run(inputs, trace=False, **spmd_kwargs):
    x = np.asarray(inputs["x"])
    b, c, h, w = x.shape
    assert (b, c, h, w) == (B_GLOBAL, C_IN, H, W), x.shape
    wA = np.asarray(inputs["wA"], np.float32)
    wB = np.asarray(inputs["wB"], np.float32)
    wV = np.asarray(inputs["wV"], np.float32)
    wR = np.asarray(inputs["wR"], np.float32)
    bA = np.asarray(inputs["bA"], np.float32)
    bB = np.asarray(inputs["bB"], np.float32)
    bV = np.asarray(inputs["bV"], np.float32)
    bR = np.asarray(inputs["bR"], np.float32)

    has_bias_abv = bool(np.any(bA) or np.any(bB) or np.any(bV))
    has_bias_r = bool(np.any(bR))
    nc = _get_nc(has_bias_abv, has_bias_r)

    # [KC, 128, 256] : chunk k holds [wA.T | wB.T][k*128:(k+1)*128, :]
    wab = np.concatenate([wA.T, wB.T], axis=1).reshape(KC, 128, 256)
    # [KC, 128, 128] : chunk k holds wV.T[k*128:(k+1)*128, :]
    wvt = wV.T.reshape(KC, 128, 128)
    base = {
        "wab": np.ascontiguousarray(wab, dtype=np.float16),
        "wvt": np.ascontiguousarray(wvt, dtype=np.float16),
        "wrt": np.ascontiguousarray(wR.T, dtype=np.float16),
    }
    if has_bias_abv:
        base["bab"] = np.concatenate([bA, bB])[None, :].astype(np.float16)
        base["bvt"] = np.ascontiguousarray(bV[:, None], np.float32)
    if has_bias_r:
        base["brt"] = np.ascontiguousarray(bR.reshape(KC, 128).T, np.float32)

    # pack x: [B, 512, 4096] -> [B, g, p, k, n']  (4KB contiguous rows)
    xh = (
        np.asarray(x, np.float16)
        .reshape(B_GLOBAL, KC, 128, NG, 512)
        .transpose(0, 3, 2, 1, 4)
    )
    in_maps = [
        dict(base, x=np.ascontiguousarray(
            xh[ci * B_LOC:(ci + 1) * B_LOC]).reshape(B_LOC, NG, 128, KC, 512))
        for ci in range(N_CORES)
    ]
    res = run_bass_kernel_spmd(
        nc, in_maps, core_ids=list(range(N_CORES)), trace=trace, **spmd_kwargs
    )
    # unpack out: [B_LOC, g, p, k, n'] -> [B_LOC, 512, 4096]
    outs = []
    for ci in range(N_CORES):
        o = res.results[ci]["out"].astype(np.float32)
        outs.append(o.transpose(0, 3, 2, 1, 4).reshape(B_LOC, C_IN, N))
    out = np.concatenate(outs, axis=0)
    return out.reshape(B_GLOBAL, C_IN, H, W), res


def kernel(**inputs):
    out, _ = _run(inputs)
    return out


# revision 10
# speedup vs baseline: 1.5728x; 1.5728x over previous
"""DoubleAttention forward on 8 Trainium2 NeuronCores.

Reference (per sample, x: [512, 4096] after flattening h*w):
    A = wA @ x + bA            [128, n]
    B = wB @ x + bB            [128, n]
    V = wV @ x + bV            [128, n]
    M = softmax(B, axis=ch)    [128, n]
    W = softmax(V, axis=ch)    [128, n]
    gd = A @ M.T               [128, 128]
    Z = gd @ W                 [128, n]
    out = wR @ Z + bR          [512, n]

Sharding: data-parallel over batch, 16 samples -> 8 cores x 2 each.

Schedule (all matmuls fp16; PE stream floor ~62us at 2.4GHz, which is
the binding constraint -- DMA is ~50us, elementwise ~45us/engine):
  - work is organized in n-chunks of 512 positions (4 pos-tiles):
    per chunk one [128,1024] PSUM quad holds A|B for 4 tiles, evacuated
    by ONE wide exp-ACT (scalar), ONE reduce (DVE), one A-scale (DVE):
    wide ops amortize the 150-450ns fixed per-op engine overheads.
  - V^T computed directly in [ch, pos] layout (weight chunks stationary)
    so phase 3 needs NO PE transposes: out = G @ Wn.  Softmax-V partition
    sums via ones128 matmul (sums replicated across partitions),
    reciprocal_approx_fast (5x faster than reciprocal), normalize on the
    otherwise-idle gpsimd (the only PSUM-free elementwise op here --
    GPSIMD cannot touch PSUM on TRN2).
  - softmax-B folded into the A-quad evacuation (per-partition scale).
  - gd accumulates on PE across chunks (1-chunk emission lag).
  - out = (wR gd) @ Wn: G^T = gd^T @ wR^T, then 32 512-col matmuls per
    sample rotating over 4 PSUM slots; evacuation split scalar/DVE.
  - DMA: host pre-packs x/out so every transfer is [128 part x 4KB
    contiguous].  Loads up-front on the sync HWDGE queue (first chunk
    on the scalar queue, in parallel); stores per-chunk on sync.
  - startup: dummy matmuls + a dummy exp during the ~6us framework
    prologue hold the PE p-state ramp (0.65->1.2->2.4GHz after 3us
    continuous busy) and preload the scalar EXP table off the
    critical path.
  - samples interleaved: s1 phase-1 chunks pad s0's gd tail and s0's
    out-phase evacuation drain.
"""

import sys

if "/opt/trn_rl_repo" not in sys.path:
    sys.path.insert(0, "/opt/trn_rl_repo")

import numpy as np

import concourse.bacc as bacc
import concourse.tile as tile
from concourse import mybir
from concourse.bass_utils import run_bass_kernel_spmd

N_CORES = 8
B_GLOBAL = 16
B_LOC = B_GLOBAL // N_CORES
C_IN, C_M, C_N = 512, 128, 128
H = W = 64
N = H * W                      # 4096 spatial positions
KC = C_IN // 128               # 4 contraction chunks
NG = 8                         # 8 n-chunks of 512 positions
SHIFT = float(-12.0 * np.log(2.0))   # exp downshift so fp16 never overflows
F32 = mybir.dt.float32
F16 = mybir.dt.float16
EXP = mybir.ActivationFunctionType.Exp
IDENT = mybir.ActivationFunctionType.Identity
AXX = mybir.AxisListType.X


def _build(has_bias_abv: bool, has_bias_r: bool):
    nc = bacc.Bacc("TRN2", target_bir_lowering=False, debug=False)

    # x / out packed host-side as [s, g, p, k, n']: 4KB-contiguous rows.
    x_d = nc.dram_tensor("x", (B_LOC, NG, 128, KC, 512), F16, kind="ExternalInput")
    # weights packed so partition rows are contiguous too
    wab_d = nc.dram_tensor("wab", (128, KC, 256), F16, kind="ExternalInput")
    wvt_d = nc.dram_tensor("wvt", (128, KC, 128), F16, kind="ExternalInput")
    wrt_d = nc.dram_tensor("wrt", (128, C_IN), F16, kind="ExternalInput")
    if has_bias_abv:
        bab_d = nc.dram_tensor("bab", (1, 256), F16, kind="ExternalInput")
        bvt_d = nc.dram_tensor("bvt", (128, 1), F32, kind="ExternalInput")
    if has_bias_r:
        brt_d = nc.dram_tensor("brt", (128, KC), F32, kind="ExternalInput")
    out_d = nc.dram_tensor("out", (B_LOC, NG, 128, KC, 512), F16, kind="ExternalOutput")

    with tile.TileContext(nc) as tc:
        with (
            tc.tile_pool(name="const", bufs=1) as constp,
            tc.tile_pool(name="xq", bufs=B_LOC * NG) as xqp,
            tc.tile_pool(name="mwb", bufs=5) as mwp,
            tc.tile_pool(name="at", bufs=5) as atp,
            tc.tile_pool(name="st", bufs=6) as stp,
            tc.tile_pool(name="ev", bufs=2) as evp,
            tc.tile_pool(name="wn", bufs=2) as wnp,
            tc.tile_pool(name="rw", bufs=4) as rwp,
            tc.tile_pool(name="gds", bufs=2) as gdsp,
            tc.tile_pool(name="gts", bufs=2) as gtsp,
            tc.tile_pool(name="osb", bufs=4) as osbp,
            tc.tile_pool(name="pP", bufs=2, space="PSUM") as pP,
            tc.tile_pool(name="pV", bufs=1, space="PSUM") as pV,
            tc.tile_pool(name="pG", bufs=1, space="PSUM") as pG,
            tc.tile_pool(name="pO", bufs=2, space="PSUM") as pO,
        ):
            # ---- warmup consts (no DMA deps) ----
            ones128 = constp.tile([128, 128], F16)
            nc.gpsimd.memset(ones128[:], 1.0)
            shiftv = constp.tile([128, 1], F32)
            nc.gpsimd.memset(shiftv[:], SHIFT)
            warm16 = constp.tile([128, 512], F16)
            nc.gpsimd.memset(warm16[:], 0.0)

            # ---- loads: first chunk on scalar queue, rest on sync ----
            xqs = [[None] * NG for _ in range(B_LOC)]

            def load_chunk(s, g, eng):
                t = xqp.tile([128, KC, 512], F16, tag="xq", name=f"xq{s}_{g}")
                eng.dma_start(t[:], x_d[s, g])
                xqs[s][g] = t

            load_chunk(0, 0, nc.scalar)
            wab = constp.tile([128, KC, 256], F16)
            nc.sync.dma_start(wab[:], wab_d[:])
            wvt = constp.tile([128, KC, 128], F16)
            nc.sync.dma_start(wvt[:], wvt_d[:])
            load_chunk(0, 1, nc.scalar)
            wrt = constp.tile([128, C_IN], F16)
            nc.sync.dma_start(wrt[:], wrt_d[:])
            for g in range(2, NG):
                load_chunk(0, g, nc.sync)
            for g in range(NG):
                load_chunk(1, g, nc.sync)

            if has_bias_abv:
                bab = constp.tile([1, 256], F16)
                nc.sync.dma_start(bab[:], bab_d[:])
                ones1 = constp.tile([1, 128], F16)
                nc.gpsimd.memset(ones1[:], 1.0)
                bvt = constp.tile([128, 1], F32)
                nc.sync.dma_start(bvt[:], bvt_d[:])
                shiftbv = constp.tile([128, 1], F32)
                nc.vector.tensor_scalar_add(shiftbv[:], bvt[:], SHIFT)
                vbias = shiftbv
            else:
                vbias = shiftv
            if has_bias_r:
                brt = constp.tile([128, KC], F32)
                nc.sync.dma_start(brt[:], brt_d[:])

            # ---- PE p-state warmup + EXP table preload during the DMA
            # wait: ~14 x 512-col dummy matmuls (~3us at mid p-state)
            warmp = pG.tile([128, 512], F32, tag="pg", name="warmp")
            for _ in range(14):
                nc.tensor.matmul(warmp[:], ones128[:], warm16[:],
                                 start=True, stop=True)
            wexp = constp.tile([128, 1], F16)
            nc.scalar.activation(wexp[:], shiftv[:], EXP, bias=0.0)

            # per-sample state
            mwB = [[None] * NG, [None] * NG]   # exp(B) fp16 [128, 4, 128]
            ats = [[None] * NG, [None] * NG]   # A/sumB fp16 [128, 4, 128]
            evt = [None, None]                 # exp(V^T) fp16 [128, NG, 512]
            wnt = [None, None]                 # normalized W^T fp16
            gdt = [None, None]                 # gd psum
            gts = [None, None]                 # G^T fp16 [128, 512]

            def emit_quad(s, g):
                """A|B projections for the 4 tiles of chunk g + softmax-B."""
                xq = xqs[s][g]
                p1 = pP.tile([128, 1024], F32, tag="pp", name=f"pp{s}_{g}")
                p1v = p1.rearrange("p (t a c) -> p t a c", t=4, c=128)
                for t in range(4):
                    dst = p1[:, t * 256:t * 256 + 256]
                    for k in range(KC):
                        nc.tensor.matmul(
                            dst, xq[:, k, t * 128:t * 128 + 128], wab[:, k, :],
                            start=(k == 0),
                            stop=(k == KC - 1 and not has_bias_abv),
                        )
                    if has_bias_abv:
                        nc.tensor.matmul(dst, ones1[:], bab[:],
                                         start=False, stop=True)
                # one wide exp + one wide reduce + one wide A-scale
                mw = mwp.tile([128, 4, 128], F16, tag="mwb", name=f"mw{s}_{g}")
                nc.scalar.activation(mw[:], p1v[:, :, 1, :], EXP, bias=shiftv[:])
                sums = stp.tile([128, 4, 1], F32, tag="sums")
                nc.vector.tensor_reduce(sums[:], mw[:], axis=AXX,
                                        op=mybir.AluOpType.add)
                rec = stp.tile([128, 4, 1], F32, tag="rec")
                nc.vector.reciprocal(rec[:], sums[:])
                at = atp.tile([128, 4, 128], F16, tag="at", name=f"at{s}_{g}")
                nc.vector.tensor_mul(at[:], p1v[:, :, 0, :],
                                     rec[:].broadcast_to([128, 4, 128]))
                mwB[s][g] = mw
                ats[s][g] = at

            def emit_vt(s, g):
                """V^T [ch, pos] projection for n-chunk g + exp."""
                pv = pV.tile([128, 512], F32, tag="pv", name=f"pv{s}_{g}")
                for k in range(KC):
                    nc.tensor.matmul(pv[:], wvt[:, k, :], xqs[s][g][:, k, :],
                                     start=(k == 0), stop=(k == KC - 1))
                nc.scalar.activation(evt[s][:, g, :], pv[:], EXP, bias=vbias[:])

            def emit_vsum(s, g):
                """Partition sums of exp(V^T) via ones-matmul; recip; Wn."""
                ps = pO.tile([128, 512], F32, tag="po", name=f"ps{s}_{g}")
                nc.tensor.matmul(ps[:], ones128[:], evt[s][:, g, :],
                                 start=True, stop=True)
                rw = rwp.tile([128, 512], F32, tag="rw", name=f"rw{s}_{g}")
                nc.vector.reciprocal_approx_fast(rw[:], ps[:])
                # fp16 SBUF-only multiply on the otherwise-idle gpsimd
                nc.gpsimd.tensor_mul(wnt[s][:, g, :], evt[s][:, g, :], rw[:])

            def emit_gd(s, g, first, last):
                for t in range(4):
                    nc.tensor.matmul(
                        gdt[s][:], ats[s][g][:, t, :], mwB[s][g][:, t, :],
                        start=(first and t == 0), stop=(last and t == 3),
                        skip_group_check=True,
                    )

            def emit_chunk(s, g):
                """All phase-1 work tied to x chunk g of sample s."""
                if g == 0:
                    evt[s] = evp.tile([128, NG, 512], F16, tag="ev",
                                      name=f"ev{s}")
                    wnt[s] = wnp.tile([128, NG, 512], F16, tag="wn",
                                      name=f"wn{s}")
                emit_quad(s, g)
                emit_vt(s, g)
                if g >= 1:
                    emit_vsum(s, g - 1)
                if g >= 2:
                    if g == 2:
                        gdt[s] = pG.tile([128, 128], F32, tag="pg",
                                         name=f"gd{s}")
                    emit_gd(s, g - 2, first=(g == 2), last=False)

            def emit_tail(s):
                """Finish gd, compute G^T = gd^T @ wR^T."""
                emit_vsum(s, NG - 1)
                emit_gd(s, NG - 2, first=False, last=False)
                emit_gd(s, NG - 1, first=False, last=True)
                gdts = gdsp.tile([128, 128], F16, tag="gds", name=f"gds{s}")
                nc.vector.tensor_copy(gdts[:], gdt[s][:])
                gtp = pO.tile([128, 512], F32, tag="po", name=f"gtp{s}")
                nc.tensor.matmul(gtp[:], gdts[:], wrt[:], start=True, stop=True)
                gt = gtsp.tile([128, 512], F16, tag="gts", name=f"gt{s}")
                nc.scalar.copy(gt[:], gtp[:])
                gts[s] = gt

            def emit_out(s, gs):
                """out chunks gs: 4 c-block matmuls + scalar/DVE-split
                evacuation + store per n-chunk.  While s=0 the second
                sample's gd accumulator owns pG, so rotate over pO/pV
                only; the final phase (s=1) can use pG as a 4th slot."""
                slots = (((pO, "po"), (pV, "pv"), (pO, "po"), (pV, "pv"))
                         if s == 0 else
                         ((pO, "po"), (pV, "pv"), (pO, "po"), (pG, "pg")))
                for g in gs:
                    osb = osbp.tile([128, KC, 512], F16, tag="osb",
                                    name=f"osb{s}_{g}")
                    for b in range(KC):
                        pool, ptag = slots[b]
                        ock = pool.tile([128, 512], F32, tag=ptag,
                                        name=f"ock{s}_{g}_{b}")
                        nc.tensor.matmul(ock[:], gts[s][:, b * 128:b * 128 + 128],
                                         wnt[s][:, g, :], start=True, stop=True)
                        dst = osb[:, b, :]
                        if has_bias_r:
                            nc.scalar.activation(dst, ock[:], IDENT,
                                                 bias=brt[:, b:b + 1])
                        elif b < 2:
                            nc.scalar.copy(dst, ock[:])
                        else:
                            nc.vector.tensor_copy(dst, ock[:])
                    nc.sync.dma_start(out_d[s, g], osb[:])

            # ---- schedule: s1 phase-1 interleaves s0's out phase ----
            for g in range(NG):
                emit_chunk(0, g)
            emit_chunk(1, 0)          # pads s0's gd-tail wait
            emit_tail(0)
            emit_chunk(1, 1)          # pads s0's G -> gts chain
            for g in range(2, NG):
                emit_out(0, [g - 2])
                emit_chunk(1, g)
            emit_out(0, [NG - 2])
            emit_tail(1)
            emit_out(0, [NG - 1])
            emit_out(1, range(NG))

    nc.compile()
    return nc


_CACHE = {}


def _get_nc(has_bias_abv: bool, has_bias_r: bool):
    key = (has_bias_abv, has_bias_r)
    if key not in _CACHE:
        _CACHE[key] = _build(*key)
    return _CACHE[key]


def _run(inputs, trace=False, **spmd_kwargs):
    x = np.asarray(inputs["x"])
    b, c, h, w = x.shape
    assert (b, c, h, w) == (B_GLOBAL, C_IN, H, W), x.shape
    wA = np.asarray(inputs["wA"], np.float32)
    wB = np.asarray(inputs["wB"], np.float32)
    wV = np.asarray(inputs["wV"], np.float32)
    wR = np.asarray(inputs["wR"], np.float32)
    bA = np.asarray(inputs["bA"], np.float32)
    bB = np.asarray(inputs["bB"], np.float32)
    bV = np.asarray(inputs["bV"], np.float32)
    bR = np.asarray(inputs["bR"], np.float32)

    has_bias_abv = bool(np.any(bA) or np.any(bB) or np.any(bV))
    has_bias_r = bool(np.any(bR))
    nc = _get_nc(has_bias_abv, has_bias_r)

    # [128, KC, 256]: row p, chunk k holds [wA.T | wB.T][k*128+p, :]
    wab = np.concatenate([wA.T, wB.T], axis=1).reshape(KC, 128, 256)
    wvt = wV.T.reshape(KC, 128, 128)
    base = {
        "wab": np.ascontiguousarray(wab.transpose(1, 0, 2), dtype=np.float16),
        "wvt": np.ascontiguousarray(wvt.transpose(1, 0, 2), dtype=np.float16),
        "wrt": np.ascontiguousarray(wR.T, dtype=np.float16),
    }
    if has_bias_abv:
        base["bab"] = np.concatenate([bA, bB])[None, :].astype(np.float16)
        base["bvt"] = np.ascontiguousarray(bV[:, None], np.float32)
    if has_bias_r:
        base["brt"] = np.ascontiguousarray(bR.reshape(KC, 128).T, np.float32)

    # pack x: [B, 512, 4096] -> [B, g, p, k, n']  (4KB contiguous rows)
    xh = (
        np.asarray(x, np.float16)
        .reshape(B_GLOBAL, KC, 128, NG, 512)
        .transpose(0, 3, 2, 1, 4)
    )
    in_maps = [
        dict(base, x=np.ascontiguousarray(
            xh[ci * B_LOC:(ci + 1) * B_LOC]))
        for ci in range(N_CORES)
    ]
    res = run_bass_kernel_spmd(
        nc, in_maps, core_ids=list(range(N_CORES)), trace=trace, **spmd_kwargs
    )
    # unpack out: [B_LOC, g, p, k, n'] -> [B_LOC, 512, 4096]
    outs = []
    for ci in range(N_CORES):
        o = res.results[ci]["out"].astype(np.float32)
        outs.append(o.transpose(0, 3, 2, 1, 4).reshape(B_LOC, C_IN, N))
    out = np.concatenate(outs, axis=0)
    return out.reshape(B_GLOBAL, C_IN, H, W), res


def kernel(**inputs):
    out, _ = _run(inputs)
    return out
